# revision 40
# baseline (speedup 1.0000x reference)
"""Trainium2 Bass kernel for causal multi-head attention with adaptive
temperature (entropy-polynomial) softmax.

Problem shape: x [2, 2048, 1024], 16 heads x 64 dims, causal.
  q/k/v = x @ W{q,k,v}.T ; sim = q k^T / 8 (causal) ;
  attn = softmax(beta * sim), beta = f(entropy(softmax(sim))) ;
  out = (attn v) @ Wo.T + bo

Sharding (8 cores): core c owns batch b = c // 4 and heads
4*(c%4) .. 4*(c%4)+3.  Host sums the 4 partials per batch + bo.

v4 structure - software-pipelined over 512-row groups t so the PE stays
continuously busy (TRN2 HAM clock gate halves the PE clock when idle):

  phase A : qT/kT = (W slice) @ x^T in f32r, drained to bf16
            (qT pre-scaled 1/8); v -> v_aug [128, 4*65] bf16 with a
            ones column per head (folds Z2 into the AV matmul).
  B1 unit (rb, h): row-wise scores in <=1024-wide chunks -> exp (ACT)
            -> causal diag zeroed on the exp VALUES via gpsimd
            affine_select -> Z1 (ACT accum for non-diag chunks, DVE
            reduce for diag chunks) ; D via DVE stt accum.
  stats(t): H = ln Z1 - D/Z1 ; beta = where(H>.5, max(poly(H),1), 1)
            on [128, 16] unit slices; beta folded into qbT = qT * beta
            (stride-0 replicate + PE transpose + DVE mult).
  B2(t, h): TRANSPOSED rescore psT[j, r] = kT^T qb in jt-pairs ->
            exp -> t2 bf16 -> diag zeroed (gpsimd) ->
            AV: avp[65, 512] += v_aug^T t2  (row 64 = Z2) ->
            attT = avp[0:64] * bcast(1/avp[64]).
  C(t)    : partial[r, o] = sum_m attT[m]^T woS[m]  (bf16)

  Main loop: for t: { B2(t, h) + B1 units of group t+1 interleaved;
  stats(t+1); C(t) } - PE alternates B2/B1 matmuls without gaps while
  ACT/DVE/Pool drain the elementwise chains behind it.
"""

import numpy as np

import concourse.bass as bass
import concourse.tile as tile
from concourse import bacc, mybir
from concourse.bass_utils import run_bass_kernel_spmd
from concourse.masks import make_identity

F32 = mybir.dt.float32
F32R = mybir.dt.float32r
BF16 = mybir.dt.bfloat16
I32 = mybir.dt.int32
AFT = mybir.ActivationFunctionType
ALU = mybir.AluOpType

B, N, DIM = 2, 2048, 1024
H_TOT, HD = 16, 64
N_CORES = 8
NH = 4            # heads per core
CD = NH * HD      # 256 channel dims per core
NRB = N // 128    # 16 row blocks
NU = NRB * NH     # 64 (rb, head) units
NT = N // 512     # 4 row-groups of 512 rows
POLY = [-0.037, 0.481, -2.3, 4.917, -1.791]
SCALE = 1.0 / 8.0  # 1/sqrt(64)


def build_kernel():
    nc = bacc.Bacc("TRN2", target_bir_lowering=False, debug=False,
                   num_devices=N_CORES)

    xT = nc.dram_tensor("xT", [DIM, N], F32, kind="ExternalInput").ap()
    wqT = nc.dram_tensor("wqT", [DIM, CD], F32, kind="ExternalInput").ap()
    wkT = nc.dram_tensor("wkT", [DIM, CD], F32, kind="ExternalInput").ap()
    wvT = nc.dram_tensor("wvT", [DIM, CD], F32, kind="ExternalInput").ap()
    woT = nc.dram_tensor("woT", [CD, DIM], F32, kind="ExternalInput").ap()
    partial = nc.dram_tensor("partial", [N, DIM], F32, kind="ExternalOutput").ap()

    KC = DIM // 128  # 8 contraction chunks

    with tile.TileContext(nc) as tc:
        # ---- persistent pools ----
        with tc.tile_pool(name="const", bufs=1) as constp, \
             tc.tile_pool(name="qkv_sb", bufs=1) as qkvp, \
             tc.tile_pool(name="attn_out", bufs=1) as aop, \
             tc.tile_pool(name="wo_sb", bufs=1) as wop, \
             tc.tile_pool(name="statsall", bufs=1) as sap, \
             tc.tile_pool(name="xw_sb", bufs=1) as xwp:

            identF = constp.tile([128, 128], F32)
            make_identity(nc, identF[:])
            ones64 = constp.tile([128, NU], F32)
            nc.vector.memset(ones64[:], 1.0)
            identB = constp.tile([128, 128], BF16)
            make_identity(nc, identB[:])
            # row mask: mask[r, j] = -1e30 if j > r else 0 (bf16)
            maskB = constp.tile([128, 128], BF16)
            nc.gpsimd.memset(maskB[:], 0.0)
            nc.gpsimd.affine_select(
                out=maskB[:], in_=maskB[:], compare_op=ALU.is_ge,
                fill=-1e30, base=0, pattern=[[-1, 128]],
                channel_multiplier=1)
            # transposed mask: maskT[j, r] = -1e30 if j > r else 0 (bf16)
            maskTB = constp.tile([128, 128], BF16)
            nc.gpsimd.memset(maskTB[:], 0.0)
            nc.gpsimd.affine_select(
                out=maskTB[:], in_=maskTB[:], compare_op=ALU.is_ge,
                fill=-1e30, base=0, pattern=[[1, 128]],
                channel_multiplier=-1)

            # persistent activations (bf16)
            qT = [qkvp.tile([128, N], BF16, tag=f"qT{m}", name=f"qT{m}") for m in range(2)]
            kT = [qkvp.tile([128, N], BF16, tag=f"kT{m}", name=f"kT{m}") for m in range(2)]
            qbT = [qkvp.tile([128, N], BF16, tag=f"qbT{m}", name=f"qbT{m}") for m in range(2)]
            v_aug = [qkvp.tile([128, NH * 65], BF16, tag=f"v{j}", name=f"v{j}")
                     for j in range(NRB)]
            attT = [aop.tile([128, N], BF16, tag=f"attT{m}", name=f"attT{m}") for m in range(2)]
            woS = [wop.tile([128, DIM], BF16, tag=f"wo{m}", name=f"wo{m}") for m in range(2)]

            Z1p = sap.tile([128, 2 * NU], F32)
            D1p = sap.tile([128, 2 * NU], F32)
            Z1a = sap.tile([128, NU], F32)
            D1a = sap.tile([128, NU], F32)
            beta_all = sap.tile([128, NU], F32)
            st_rz = sap.tile([128, NU], F32)
            st_dn = sap.tile([128, NU], F32)
            st_ln = sap.tile([128, NU], F32)
            st_H = sap.tile([128, NU], F32)
            st_p0 = sap.tile([128, NU], F32)
            st_p1 = sap.tile([128, NU], F32)
            st_mk = sap.tile([128, NU], I32)
            st_mi = sap.tile([128, NU], I32)
            st_ei = sap.tile([128, NU], I32)
            st_ef = sap.tile([128, NU], F32)

            # ---- phase A: QKV projections (f32r in, bf16 out) ----
            with tc.tile_pool(name="qkv_ps", bufs=4, space="PSUM") as qkps:
                xTs = [xwp.tile([128, N], F32R, tag=f"xT{k}", name=f"xTs{k}") for k in range(KC)]
                wq_s = [xwp.tile([128, CD], F32R, tag=f"wq{k}", name=f"wq{k}") for k in range(KC)]
                wk_s = [xwp.tile([128, CD], F32R, tag=f"wk{k}", name=f"wk{k}") for k in range(KC)]
                wv_s = [xwp.tile([128, CD], F32R, tag=f"wv{k}", name=f"wv{k}") for k in range(KC)]
                woF = [xwp.tile([128, DIM], F32, tag=f"woF{m}", name=f"woF{m}") for m in range(2)]
                # q/k inputs first so the first projections start early
                for k in range(KC):
                    sl = slice(128 * k, 128 * (k + 1))
                    nc.sync.dma_start(wq_s[k][:], wqT[sl, :].bitcast(F32R))
                    nc.sync.dma_start(wk_s[k][:], wkT[sl, :].bitcast(F32R))
                    nc.sync.dma_start(xTs[k][:], xT[sl, :].bitcast(F32R))
                for k in range(KC):
                    sl = slice(128 * k, 128 * (k + 1))
                    nc.sync.dma_start(wv_s[k][:], wvT[sl, :].bitcast(F32R))
                for m in range(2):
                    nc.sync.dma_start(woF[m][:], woT[128 * m:128 * (m + 1), :])
                    nc.gpsimd.tensor_copy(woS[m][:], woF[m][:])

                for m in range(2):
                    for which, wt, dest, scl in (("q", wq_s, qT, SCALE), ("k", wk_s, kT, 1.0)):
                        for nn in range(N // 512):
                            pq = qkps.tile([128, 512], F32, tag="pq")
                            for k in range(KC):
                                nc.tensor.matmul(
                                    pq[:], wt[k][:, 128 * m:128 * (m + 1)],
                                    xTs[k][:, 512 * nn:512 * (nn + 1)],
                                    start=(k == 0), stop=(k == KC - 1))
                            nc.scalar.activation(
                                dest[m][:, 512 * nn:512 * (nn + 1)], pq[:],
                                AFT.Copy, bias=0.0, scale=scl)

            nc.vector.memset(Z1p[:], 0.0)
            nc.vector.memset(D1p[:], 0.0)

            # ---- pipelined B1 / stats / B2 / C ----
            with tc.tile_pool(name="sc_ps", bufs=3, space="PSUM") as scp, \
                 tc.tile_pool(name="av_ps", bufs=2, space="PSUM") as avpp, \
                 tc.tile_pool(name="t1p", bufs=3) as t1p, \
                 tc.tile_pool(name="scr2", bufs=2) as scr2, \
                 tc.tile_pool(name="t2p", bufs=3) as t2p, \
                 tc.tile_pool(name="rzp", bufs=2) as rzp, \
                 tc.tile_pool(name="bcp", bufs=4) as bcp, \
                 tc.tile_pool(name="ostp", bufs=3) as ostp:

                def emit_v(jt):
                    pv = scp.tile([128, 1024], F32, tag="sc")
                    for k in range(KC):
                        nc.tensor.matmul(
                            pv[:, 0:CD], xTs[k][:, 128 * jt:128 * (jt + 1)],
                            wv_s[k][:], start=(k == 0), stop=(k == KC - 1))
                    nc.gpsimd.memset(v_aug[jt][:], 1.0)
                    nc.vector.tensor_copy(
                        v_aug[jt].rearrange("p (h d) -> p h d", d=65)[:, :, 0:64],
                        pv[:, 0:CD].rearrange("p (h d) -> p h d", d=64))

                def emit_b1_unit(rb, h):
                    """Row-wise stat sweep for unit (rb, h): Z1, D."""
                    u = rb * NH + h
                    m, base = h // 2, 64 * (h % 2)
                    q_l = qT[m][base:base + 64, 128 * rb:128 * (rb + 1)]
                    W = 128 * (rb + 1)
                    for ci, off in enumerate(range(0, W, 1024)):
                        cw = min(1024, W - off)
                        has_diag = off + cw == W
                        ps = scp.tile([128, 1024], F32, tag="sc")
                        for o2 in range(0, cw, 512):
                            sw = min(512, cw - o2)
                            last = has_diag and o2 + sw == cw
                            nc.tensor.matmul(
                                ps[:, o2:o2 + sw], q_l,
                                kT[m][base:base + 64, off + o2:off + o2 + sw],
                                start=True, stop=not last,
                                skip_group_check=last)
                        if has_diag:
                            nc.tensor.matmul(
                                ps[:, cw - 128:cw], identB[:], maskB[:],
                                start=False, stop=True, skip_group_check=True)
                        t1 = t1p.tile([128, 1024], BF16, tag="t1")
                        if has_diag:
                            nc.scalar.activation(
                                t1[:, :cw], ps[:, :cw], AFT.Exp,
                                bias=0.0, scale=1.0)
                            nc.vector.tensor_reduce(
                                out=Z1p[:, 2 * u + ci:2 * u + ci + 1],
                                in_=t1[:, :cw],
                                axis=mybir.AxisListType.X, op=ALU.add)
                        else:
                            nc.scalar.activation(
                                t1[:, :cw], ps[:, :cw], AFT.Exp,
                                bias=0.0, scale=1.0,
                                accum_out=Z1p[:, 2 * u + ci:2 * u + ci + 1])
                        s2 = scr2.tile([128, 1024], BF16, tag="s2")
                        nc.vector.scalar_tensor_tensor(
                            out=s2[:, :cw], in0=ps[:, :cw], scalar=1.0,
                            in1=t1[:, :cw], op0=ALU.mult, op1=ALU.mult,
                            accum_out=D1p[:, 2 * u + ci:2 * u + ci + 1])

                def emit_stats_qb(t):
                    """beta for units of row-group t, fold into qbT."""
                    us = slice(16 * t, 16 * (t + 1))
                    nc.vector.tensor_reduce(
                        out=Z1a[:, us],
                        in_=Z1p[:, 32 * t:32 * (t + 1)].rearrange(
                            "p (u c) -> p u c", c=2),
                        axis=mybir.AxisListType.X, op=ALU.add)
                    nc.vector.tensor_reduce(
                        out=D1a[:, us],
                        in_=D1p[:, 32 * t:32 * (t + 1)].rearrange(
                            "p (u c) -> p u c", c=2),
                        axis=mybir.AxisListType.X, op=ALU.add)
                    nc.vector.reciprocal(st_rz[:, us], Z1a[:, us])
                    nc.vector.tensor_mul(st_dn[:, us], D1a[:, us], st_rz[:, us])
                    # ln(Z1) on DVE: exponent/mantissa split + deg-6 poly
                    zb = Z1a[:, us].bitcast(I32)
                    nc.vector.tensor_scalar(
                        out=st_mi[:, us], in0=zb, scalar1=0x7FFFFF,
                        scalar2=0x3F800000, op0=ALU.bitwise_and,
                        op1=ALU.bitwise_or)
                    nc.vector.tensor_scalar(
                        out=st_ei[:, us], in0=zb, scalar1=23, scalar2=None,
                        op0=ALU.logical_shift_right)
                    nc.vector.tensor_copy(st_ef[:, us], st_ei[:, us])
                    mant = st_mi[:, us].bitcast(F32)
                    LNC = [-0.017208480121667386, 0.18497955009451103,
                           -0.8555561826105713, 2.231191545727832,
                           -3.6488845206006824, 4.20456481831467,
                           -2.099083228802664]
                    nc.vector.tensor_scalar(
                        out=st_p0[:, us], in0=mant, scalar1=LNC[0],
                        scalar2=LNC[1], op0=ALU.mult, op1=ALU.add)
                    for c in LNC[2:]:
                        nc.vector.tensor_mul(st_p1[:, us], st_p0[:, us], mant)
                        nc.vector.tensor_scalar_add(st_p0[:, us],
                                                    st_p1[:, us], c)
                    nc.vector.tensor_scalar(
                        out=st_ef[:, us], in0=st_ef[:, us],
                        scalar1=0.6931471805599453,
                        scalar2=-88.02969193111305, op0=ALU.mult, op1=ALU.add)
                    nc.vector.tensor_tensor(
                        out=st_ln[:, us], in0=st_p0[:, us],
                        in1=st_ef[:, us], op=ALU.add)
                    nc.vector.tensor_sub(st_H[:, us], st_ln[:, us], st_dn[:, us])
                    nc.vector.tensor_scalar(
                        out=st_p0[:, us], in0=st_H[:, us], scalar1=POLY[0],
                        scalar2=POLY[1], op0=ALU.mult, op1=ALU.add)
                    for c in POLY[2:]:
                        nc.vector.tensor_mul(st_p1[:, us], st_p0[:, us], st_H[:, us])
                        nc.vector.tensor_scalar_add(st_p0[:, us], st_p1[:, us], c)
                    nc.vector.tensor_scalar_max(st_p1[:, us], st_p0[:, us], 1.0)
                    nc.vector.tensor_scalar(out=st_mk[:, us], in0=st_H[:, us],
                                            scalar1=0.5, scalar2=None,
                                            op0=ALU.is_gt)
                    nc.vector.tensor_copy(beta_all[:, us], ones64[:, us])
                    nc.vector.copy_predicated(beta_all[:, us], st_mk[:, us],
                                              st_p1[:, us])
                    # qbT = qT * bcast(beta)
                    for g in range(2):  # two quads of (m, rb) pairs
                        bc4t = scp.tile([128, 1024], F32, tag="sc")
                        bc4 = bc4t[:, 0:512]
                        pairs = []
                        for i in range(4):
                            idx = 4 * g + i
                            m, rb = idx % 2, 4 * t + idx // 2
                            u0 = 4 * rb + 2 * m
                            src = beta_all[:, u0:u0 + 2]
                            view = bass.AP(src.tensor, src.offset,
                                           [src.ap[0], src.ap[1], [0, 64]])
                            bcT = bcp.tile([128, 128], F32, tag="bcT")
                            nc.vector.tensor_copy(
                                bcT.rearrange("p (h r) -> p h r", r=64), view)
                            nc.tensor.transpose(
                                bc4[:, 128 * i:128 * (i + 1)], bcT[:], identF[:])
                            pairs.append((m, rb, i))
                        for m, rb, i in pairs:
                            cols = slice(128 * rb, 128 * (rb + 1))
                            nc.vector.tensor_tensor(
                                out=qbT[m][:, cols], in0=qT[m][:, cols],
                                in1=bc4[:, 128 * i:128 * (i + 1)], op=ALU.mult)

                def emit_b2_head(t, h, filler=None, pulls=1):
                    """Transposed rescore + AV + normalize for (t, h)."""
                    njt = 4 * (t + 1)
                    rcols = slice(512 * t, 512 * (t + 1))
                    m, base = h // 2, 64 * (h % 2)
                    qb_l = qbT[m][base:base + 64, rcols]
                    avp = avpp.tile([128, 512], F32, tag="avp")

                    def emit_av(j1, j2, t2, c1, c2):
                        for jj, cc, half in ((j1, c1, 0), (j2, c2, 1)):
                            lo = 128 * cc if cc > 0 else 0
                            nc.tensor.matmul(
                                avp[0:65, lo:512],
                                v_aug[jj][:, 65 * h:65 * h + 65],
                                t2[:, 512 * half + lo:512 * (half + 1)],
                                start=(jj == 0), stop=(jj == njt - 1),
                                skip_group_check=True)

                    pend = []
                    for p in range(njt // 2):
                        j1, j2 = 2 * p, 2 * p + 1
                        c1, c2 = j1 - 4 * t, j2 - 4 * t
                        psT = scp.tile([128, 1024], F32, tag="sc")
                        for jj, cc, half in ((j1, c1, 0), (j2, c2, 1)):
                            ing = cc >= 0
                            nc.tensor.matmul(
                                psT[:, 512 * half:512 * (half + 1)],
                                kT[m][base:base + 64, 128 * jj:128 * (jj + 1)],
                                qb_l, start=True, stop=not ing,
                                skip_group_check=ing)
                            if ing:
                                dg = slice(512 * half + 128 * cc,
                                           512 * half + 128 * (cc + 1))
                                nc.tensor.matmul(
                                    psT[:, dg], identB[:], maskTB[:],
                                    start=False, stop=True,
                                    skip_group_check=True)
                        lo = 128 * c1 if c1 > 0 else 0
                        t2 = t2p.tile([128, 1024], BF16, tag="t2")
                        nc.scalar.activation(
                            t2[:, lo:1024], psT[:, lo:1024], AFT.Exp,
                            bias=0.0, scale=1.0)
                        pend.append((j1, j2, t2, c1, c2))
                        if len(pend) > 2:
                            emit_av(*pend.pop(0))
                        if filler is not None:
                            for _ in range(pulls):
                                nxt = next(filler, None)
                                if nxt is not None:
                                    emit_b1_unit(*nxt)
                    for pv_ in pend:
                        emit_av(*pv_)

                    # normalize by Z2 (row 64 of avp)
                    avr = rzp.tile([128, 512], F32, tag="avr")
                    nc.vector.tensor_copy(avr[0:65, :], avp[0:65, :])
                    rz2 = rzp.tile([128, 512], F32, tag="rz2")
                    nc.vector.reciprocal(rz2[0:1, :], avr[64:65, :])
                    rbc = rzp.tile([128, 512], F32, tag="rbc")
                    nc.gpsimd.partition_broadcast(rbc[0:64, :], rz2[0:1, :])
                    nc.vector.tensor_tensor(
                        out=attT[m][base:base + 64, rcols],
                        in0=avr[0:64, :], in1=rbc[0:64, :], op=ALU.mult)

                def emit_c(t):
                    for rb in range(4 * t, 4 * t + 4):
                        for nn in range(2):
                            ppt = scp.tile([128, 1024], F32, tag="sc")
                            pp = ppt[:, 0:512]
                            for m in range(2):
                                nc.tensor.matmul(
                                    pp[:], attT[m][:, 128 * rb:128 * (rb + 1)],
                                    woS[m][:, 512 * nn:512 * (nn + 1)],
                                    start=(m == 0), stop=(m == 1))
                            ost = ostp.tile([128, 512], F32, tag="ost")
                            nc.vector.tensor_copy(ost[:], pp[:])
                            nc.sync.dma_start(
                                partial[128 * rb:128 * (rb + 1),
                                        512 * nn:512 * (nn + 1)],
                                ost[:])

                # prologue: B1 for row-group 0, v projections interleaved
                for rb in range(4):
                    for h in range(NH):
                        emit_b1_unit(rb, h)
                        emit_v(4 * rb + h)
                emit_stats_qb(0)

                for t in range(NT):
                    units = []
                    if t + 1 < NT:
                        units = [(4 * (t + 1) + j, hh)
                                 for hh in range(NH) for j in range(4)]
                    fl = iter(units)
                    for h in range(NH):
                        emit_b2_head(t, h, fl, pulls=(2 if t == 0 else 1))
                    emit_c(t)
                    for rem in fl:
                        emit_b1_unit(*rem)
                    if t + 1 < NT:
                        emit_stats_qb(t + 1)

    nc.compile()
    return nc


_NC_CACHE = None
_LAST_IN_MAPS = None


def kernel(x, Wq, Wk, Wv, Wo, bo):
    global _NC_CACHE, _LAST_IN_MAPS
    x = np.asarray(x, dtype=np.float32)
    Wq = np.asarray(Wq, dtype=np.float32)
    Wk = np.asarray(Wk, dtype=np.float32)
    Wv = np.asarray(Wv, dtype=np.float32)
    Wo = np.asarray(Wo, dtype=np.float32)
    bo = np.asarray(bo, dtype=np.float32)

    if _NC_CACHE is None:
        _NC_CACHE = build_kernel()
    nc = _NC_CACHE

    woT_full = np.ascontiguousarray(Wo.T)  # [c, o]

    in_maps = []
    for c in range(N_CORES):
        b = c // 4
        s0 = CD * (c % 4)
        sl = slice(s0, s0 + CD)
        in_maps.append({
            "xT": np.ascontiguousarray(x[b].T),
            "wqT": np.ascontiguousarray(Wq[sl, :].T),
            "wkT": np.ascontiguousarray(Wk[sl, :].T),
            "wvT": np.ascontiguousarray(Wv[sl, :].T),
            "woT": np.ascontiguousarray(woT_full[sl, :]),
        })

    _LAST_IN_MAPS = in_maps
    res = run_bass_kernel_spmd(nc, in_maps, core_ids=list(range(N_CORES)))

    out = np.zeros((B, N, DIM), dtype=np.float32)
    for c in range(N_CORES):
        out[c // 4] += res.results[c]["partial"]
    out += bo[None, None, :]
    return out


# revision 41
# speedup vs baseline: 1.0150x; 1.0150x over previous
"""Trainium2 Bass kernel for causal multi-head attention with adaptive
temperature (entropy-polynomial) softmax.

Problem shape: x [2, 2048, 1024], 16 heads x 64 dims, causal.
  q/k/v = x @ W{q,k,v}.T ; sim = q k^T / 8 (causal) ;
  attn = softmax(beta * sim), beta = f(entropy(softmax(sim))) ;
  out = (attn v) @ Wo.T + bo

Sharding (8 cores): core c owns batch b = c // 4 and heads
4*(c%4) .. 4*(c%4)+3.  Host sums the 4 partials per batch + bo.

v4 structure - software-pipelined over 512-row groups t so the PE stays
continuously busy (TRN2 HAM clock gate halves the PE clock when idle):

  phase A : qT/kT = (W slice) @ x^T in f32r, drained to bf16
            (qT pre-scaled 1/8); v -> v_aug [128, 4*65] bf16 with a
            ones column per head (folds Z2 into the AV matmul).
  B1 unit (rb, h): row-wise scores in <=1024-wide chunks -> exp (ACT)
            -> causal diag zeroed on the exp VALUES via gpsimd
            affine_select -> Z1 (ACT accum for non-diag chunks, DVE
            reduce for diag chunks) ; D via DVE stt accum.
  stats(t): H = ln Z1 - D/Z1 ; beta = where(H>.5, max(poly(H),1), 1)
            on [128, 16] unit slices; beta folded into qbT = qT * beta
            (stride-0 replicate + PE transpose + DVE mult).
  B2(t, h): TRANSPOSED rescore psT[j, r] = kT^T qb in jt-pairs ->
            exp -> t2 bf16 -> diag zeroed (gpsimd) ->
            AV: avp[65, 512] += v_aug^T t2  (row 64 = Z2) ->
            attT = avp[0:64] * bcast(1/avp[64]).
  C(t)    : partial[r, o] = sum_m attT[m]^T woS[m]  (bf16)

  Main loop: for t: { B2(t, h) + B1 units of group t+1 interleaved;
  stats(t+1); C(t) } - PE alternates B2/B1 matmuls without gaps while
  ACT/DVE/Pool drain the elementwise chains behind it.
"""

import numpy as np

import concourse.bass as bass
import concourse.tile as tile
from concourse import bacc, mybir
from concourse.bass_utils import run_bass_kernel_spmd
from concourse.masks import make_identity

F32 = mybir.dt.float32
F32R = mybir.dt.float32r
BF16 = mybir.dt.bfloat16
I32 = mybir.dt.int32
AFT = mybir.ActivationFunctionType
ALU = mybir.AluOpType

B, N, DIM = 2, 2048, 1024
H_TOT, HD = 16, 64
N_CORES = 8
NH = 4            # heads per core
CD = NH * HD      # 256 channel dims per core
NRB = N // 128    # 16 row blocks
NU = NRB * NH     # 64 (rb, head) units
NT = N // 512     # 4 row-groups of 512 rows
POLY = [-0.037, 0.481, -2.3, 4.917, -1.791]
SCALE = 1.0 / 8.0  # 1/sqrt(64)


def build_kernel():
    nc = bacc.Bacc("TRN2", target_bir_lowering=False, debug=False,
                   num_devices=N_CORES)

    xT = nc.dram_tensor("xT", [DIM, N], F32, kind="ExternalInput").ap()
    wqT = nc.dram_tensor("wqT", [DIM, CD], F32, kind="ExternalInput").ap()
    wkT = nc.dram_tensor("wkT", [DIM, CD], F32, kind="ExternalInput").ap()
    wvT = nc.dram_tensor("wvT", [DIM, CD], F32, kind="ExternalInput").ap()
    woT = nc.dram_tensor("woT", [CD, DIM], F32, kind="ExternalInput").ap()
    partial = nc.dram_tensor("partial", [N, DIM], F32, kind="ExternalOutput").ap()

    KC = DIM // 128  # 8 contraction chunks

    with tile.TileContext(nc) as tc:
        # ---- persistent pools ----
        with tc.tile_pool(name="const", bufs=1) as constp, \
             tc.tile_pool(name="qkv_sb", bufs=1) as qkvp, \
             tc.tile_pool(name="attn_out", bufs=1) as aop, \
             tc.tile_pool(name="wo_sb", bufs=1) as wop, \
             tc.tile_pool(name="statsall", bufs=1) as sap, \
             tc.tile_pool(name="xw_sb", bufs=1) as xwp:

            identF = constp.tile([128, 128], F32)
            make_identity(nc, identF[:])
            ones64 = constp.tile([128, NU], F32)
            nc.vector.memset(ones64[:], 1.0)
            identB = constp.tile([128, 128], BF16)
            make_identity(nc, identB[:])
            # row mask: mask[r, j] = -1e30 if j > r else 0 (bf16)
            maskB = constp.tile([128, 128], BF16)
            nc.gpsimd.memset(maskB[:], 0.0)
            nc.gpsimd.affine_select(
                out=maskB[:], in_=maskB[:], compare_op=ALU.is_ge,
                fill=-1e30, base=0, pattern=[[-1, 128]],
                channel_multiplier=1)
            # transposed mask: maskT[j, r] = -1e30 if j > r else 0 (bf16)
            maskTB = constp.tile([128, 128], BF16)
            nc.gpsimd.memset(maskTB[:], 0.0)
            nc.gpsimd.affine_select(
                out=maskTB[:], in_=maskTB[:], compare_op=ALU.is_ge,
                fill=-1e30, base=0, pattern=[[1, 128]],
                channel_multiplier=-1)

            # persistent activations (bf16)
            qT = [qkvp.tile([128, N], BF16, tag=f"qT{m}", name=f"qT{m}") for m in range(2)]
            kT = [qkvp.tile([128, N], BF16, tag=f"kT{m}", name=f"kT{m}") for m in range(2)]
            qbT = [qkvp.tile([128, N], BF16, tag=f"qbT{m}", name=f"qbT{m}") for m in range(2)]
            v_aug = [qkvp.tile([128, NH * 65], BF16, tag=f"v{j}", name=f"v{j}")
                     for j in range(NRB)]
            attT = [aop.tile([128, N], BF16, tag=f"attT{m}", name=f"attT{m}") for m in range(2)]
            woS = [wop.tile([128, DIM], BF16, tag=f"wo{m}", name=f"wo{m}") for m in range(2)]

            Z1p = sap.tile([128, 2 * NU], F32)
            D1p = sap.tile([128, 2 * NU], F32)
            Z1a = sap.tile([128, NU], F32)
            D1a = sap.tile([128, NU], F32)
            beta_all = sap.tile([128, NU], F32)
            st_rz = sap.tile([128, NU], F32)
            st_dn = sap.tile([128, NU], F32)
            st_ln = sap.tile([128, NU], F32)
            st_H = sap.tile([128, NU], F32)
            st_p0 = sap.tile([128, NU], F32)
            st_p1 = sap.tile([128, NU], F32)
            st_mk = sap.tile([128, NU], I32)
            st_mi = sap.tile([128, NU], I32)
            st_ei = sap.tile([128, NU], I32)
            st_ef = sap.tile([128, NU], F32)

            # ---- phase A: QKV projections (f32r in, bf16 out) ----
            with tc.tile_pool(name="qkv_ps", bufs=4, space="PSUM") as qkps:
                xTs = [xwp.tile([128, N], F32R, tag=f"xT{k}", name=f"xTs{k}") for k in range(KC)]
                wq_s = [xwp.tile([128, CD], F32R, tag=f"wq{k}", name=f"wq{k}") for k in range(KC)]
                wk_s = [xwp.tile([128, CD], F32R, tag=f"wk{k}", name=f"wk{k}") for k in range(KC)]
                wv_s = [xwp.tile([128, CD], F32R, tag=f"wv{k}", name=f"wv{k}") for k in range(KC)]
                woF = [xwp.tile([128, DIM], F32, tag=f"woF{m}", name=f"woF{m}") for m in range(2)]
                # q/k inputs first so the first projections start early
                for k in range(KC):
                    sl = slice(128 * k, 128 * (k + 1))
                    nc.sync.dma_start(wq_s[k][:], wqT[sl, :].bitcast(F32R))
                    nc.sync.dma_start(wk_s[k][:], wkT[sl, :].bitcast(F32R))
                    nc.sync.dma_start(xTs[k][:], xT[sl, :].bitcast(F32R))
                for k in range(KC):
                    sl = slice(128 * k, 128 * (k + 1))
                    nc.sync.dma_start(wv_s[k][:], wvT[sl, :].bitcast(F32R))
                for m in range(2):
                    nc.sync.dma_start(woF[m][:], woT[128 * m:128 * (m + 1), :])
                    nc.gpsimd.tensor_copy(woS[m][:], woF[m][:])

                for m in range(2):
                    for which, wt, dest, scl in (("q", wq_s, qT, SCALE), ("k", wk_s, kT, 1.0)):
                        for nn in range(N // 512):
                            pq = qkps.tile([128, 512], F32, tag="pq")
                            for k in range(KC):
                                nc.tensor.matmul(
                                    pq[:], wt[k][:, 128 * m:128 * (m + 1)],
                                    xTs[k][:, 512 * nn:512 * (nn + 1)],
                                    start=(k == 0), stop=(k == KC - 1))
                            nc.scalar.activation(
                                dest[m][:, 512 * nn:512 * (nn + 1)], pq[:],
                                AFT.Copy, bias=0.0, scale=scl)

            nc.vector.memset(Z1p[:], 0.0)
            nc.vector.memset(D1p[:], 0.0)

            # ---- pipelined B1 / stats / B2 / C ----
            with tc.tile_pool(name="sc_ps", bufs=3, space="PSUM") as scp, \
                 tc.tile_pool(name="av_ps", bufs=2, space="PSUM") as avpp, \
                 tc.tile_pool(name="t1p", bufs=3) as t1p, \
                 tc.tile_pool(name="scr2", bufs=2) as scr2, \
                 tc.tile_pool(name="t2p", bufs=3) as t2p, \
                 tc.tile_pool(name="rzp", bufs=2) as rzp, \
                 tc.tile_pool(name="bcp", bufs=4) as bcp, \
                 tc.tile_pool(name="ostp", bufs=3) as ostp:

                def ka(n):
                    for _ in range(n):
                        nc.tensor.ldweights(identB[:])

                def emit_v(jt):
                    pv = scp.tile([128, 1024], F32, tag="sc")
                    for k in range(KC):
                        nc.tensor.matmul(
                            pv[:, 0:CD], xTs[k][:, 128 * jt:128 * (jt + 1)],
                            wv_s[k][:], start=(k == 0), stop=(k == KC - 1))
                    nc.gpsimd.memset(v_aug[jt][:], 1.0)
                    nc.vector.tensor_copy(
                        v_aug[jt].rearrange("p (h d) -> p h d", d=65)[:, :, 0:64],
                        pv[:, 0:CD].rearrange("p (h d) -> p h d", d=64))

                def emit_b1_unit(rb, h):
                    """Row-wise stat sweep for unit (rb, h): Z1, D."""
                    u = rb * NH + h
                    m, base = h // 2, 64 * (h % 2)
                    q_l = qT[m][base:base + 64, 128 * rb:128 * (rb + 1)]
                    W = 128 * (rb + 1)
                    for ci, off in enumerate(range(0, W, 1024)):
                        cw = min(1024, W - off)
                        has_diag = off + cw == W
                        ps = scp.tile([128, 1024], F32, tag="sc")
                        for o2 in range(0, cw, 512):
                            sw = min(512, cw - o2)
                            last = has_diag and o2 + sw == cw
                            nc.tensor.matmul(
                                ps[:, o2:o2 + sw], q_l,
                                kT[m][base:base + 64, off + o2:off + o2 + sw],
                                start=True, stop=not last,
                                skip_group_check=last)
                        if has_diag:
                            nc.tensor.matmul(
                                ps[:, cw - 128:cw], identB[:], maskB[:],
                                start=False, stop=True, skip_group_check=True)
                        t1 = t1p.tile([128, 1024], BF16, tag="t1")
                        if has_diag:
                            nc.scalar.activation(
                                t1[:, :cw], ps[:, :cw], AFT.Exp,
                                bias=0.0, scale=1.0)
                            nc.vector.tensor_reduce(
                                out=Z1p[:, 2 * u + ci:2 * u + ci + 1],
                                in_=t1[:, :cw],
                                axis=mybir.AxisListType.X, op=ALU.add)
                        else:
                            nc.scalar.activation(
                                t1[:, :cw], ps[:, :cw], AFT.Exp,
                                bias=0.0, scale=1.0,
                                accum_out=Z1p[:, 2 * u + ci:2 * u + ci + 1])
                        s2 = scr2.tile([128, 1024], BF16, tag="s2")
                        nc.vector.scalar_tensor_tensor(
                            out=s2[:, :cw], in0=ps[:, :cw], scalar=1.0,
                            in1=t1[:, :cw], op0=ALU.mult, op1=ALU.mult,
                            accum_out=D1p[:, 2 * u + ci:2 * u + ci + 1])
                        ka(3)

                def emit_stats_qb(t):
                    """beta for units of row-group t, fold into qbT."""
                    ka(32)
                    us = slice(16 * t, 16 * (t + 1))
                    nc.vector.tensor_reduce(
                        out=Z1a[:, us],
                        in_=Z1p[:, 32 * t:32 * (t + 1)].rearrange(
                            "p (u c) -> p u c", c=2),
                        axis=mybir.AxisListType.X, op=ALU.add)
                    nc.vector.tensor_reduce(
                        out=D1a[:, us],
                        in_=D1p[:, 32 * t:32 * (t + 1)].rearrange(
                            "p (u c) -> p u c", c=2),
                        axis=mybir.AxisListType.X, op=ALU.add)
                    nc.vector.reciprocal(st_rz[:, us], Z1a[:, us])
                    nc.vector.tensor_mul(st_dn[:, us], D1a[:, us], st_rz[:, us])
                    # ln(Z1) on DVE: exponent/mantissa split + deg-6 poly
                    zb = Z1a[:, us].bitcast(I32)
                    nc.vector.tensor_scalar(
                        out=st_mi[:, us], in0=zb, scalar1=0x7FFFFF,
                        scalar2=0x3F800000, op0=ALU.bitwise_and,
                        op1=ALU.bitwise_or)
                    nc.vector.tensor_scalar(
                        out=st_ei[:, us], in0=zb, scalar1=23, scalar2=None,
                        op0=ALU.logical_shift_right)
                    nc.vector.tensor_copy(st_ef[:, us], st_ei[:, us])
                    mant = st_mi[:, us].bitcast(F32)
                    LNC = [-0.017208480121667386, 0.18497955009451103,
                           -0.8555561826105713, 2.231191545727832,
                           -3.6488845206006824, 4.20456481831467,
                           -2.099083228802664]
                    nc.vector.tensor_scalar(
                        out=st_p0[:, us], in0=mant, scalar1=LNC[0],
                        scalar2=LNC[1], op0=ALU.mult, op1=ALU.add)
                    for c in LNC[2:]:
                        nc.vector.tensor_mul(st_p1[:, us], st_p0[:, us], mant)
                        nc.vector.tensor_scalar_add(st_p0[:, us],
                                                    st_p1[:, us], c)
                    nc.vector.tensor_scalar(
                        out=st_ef[:, us], in0=st_ef[:, us],
                        scalar1=0.6931471805599453,
                        scalar2=-88.02969193111305, op0=ALU.mult, op1=ALU.add)
                    nc.vector.tensor_tensor(
                        out=st_ln[:, us], in0=st_p0[:, us],
                        in1=st_ef[:, us], op=ALU.add)
                    nc.vector.tensor_sub(st_H[:, us], st_ln[:, us], st_dn[:, us])
                    nc.vector.tensor_scalar(
                        out=st_p0[:, us], in0=st_H[:, us], scalar1=POLY[0],
                        scalar2=POLY[1], op0=ALU.mult, op1=ALU.add)
                    for c in POLY[2:]:
                        nc.vector.tensor_mul(st_p1[:, us], st_p0[:, us], st_H[:, us])
                        nc.vector.tensor_scalar_add(st_p0[:, us], st_p1[:, us], c)
                    nc.vector.tensor_scalar_max(st_p1[:, us], st_p0[:, us], 1.0)
                    nc.vector.tensor_scalar(out=st_mk[:, us], in0=st_H[:, us],
                                            scalar1=0.5, scalar2=None,
                                            op0=ALU.is_gt)
                    nc.vector.tensor_copy(beta_all[:, us], ones64[:, us])
                    nc.vector.copy_predicated(beta_all[:, us], st_mk[:, us],
                                              st_p1[:, us])
                    # qbT = qT * bcast(beta)
                    for g in range(2):  # two quads of (m, rb) pairs
                        bc4t = scp.tile([128, 1024], F32, tag="sc")
                        bc4 = bc4t[:, 0:512]
                        pairs = []
                        for i in range(4):
                            idx = 4 * g + i
                            m, rb = idx % 2, 4 * t + idx // 2
                            u0 = 4 * rb + 2 * m
                            src = beta_all[:, u0:u0 + 2]
                            view = bass.AP(src.tensor, src.offset,
                                           [src.ap[0], src.ap[1], [0, 64]])
                            bcT = bcp.tile([128, 128], F32, tag="bcT")
                            nc.vector.tensor_copy(
                                bcT.rearrange("p (h r) -> p h r", r=64), view)
                            nc.tensor.transpose(
                                bc4[:, 128 * i:128 * (i + 1)], bcT[:], identF[:])
                            pairs.append((m, rb, i))
                        for m, rb, i in pairs:
                            cols = slice(128 * rb, 128 * (rb + 1))
                            nc.vector.tensor_tensor(
                                out=qbT[m][:, cols], in0=qT[m][:, cols],
                                in1=bc4[:, 128 * i:128 * (i + 1)], op=ALU.mult)

                def emit_b2_head(t, h, filler=None, pulls=1):
                    """Transposed rescore + AV + normalize for (t, h)."""
                    njt = 4 * (t + 1)
                    rcols = slice(512 * t, 512 * (t + 1))
                    m, base = h // 2, 64 * (h % 2)
                    qb_l = qbT[m][base:base + 64, rcols]
                    avp = avpp.tile([128, 512], F32, tag="avp")

                    def emit_av(j1, j2, t2, c1, c2):
                        for jj, cc, half in ((j1, c1, 0), (j2, c2, 1)):
                            lo = 128 * cc if cc > 0 else 0
                            nc.tensor.matmul(
                                avp[0:65, lo:512],
                                v_aug[jj][:, 65 * h:65 * h + 65],
                                t2[:, 512 * half + lo:512 * (half + 1)],
                                start=(jj == 0), stop=(jj == njt - 1),
                                skip_group_check=True)

                    pend = []
                    for p in range(njt // 2):
                        j1, j2 = 2 * p, 2 * p + 1
                        c1, c2 = j1 - 4 * t, j2 - 4 * t
                        psT = scp.tile([128, 1024], F32, tag="sc")
                        for jj, cc, half in ((j1, c1, 0), (j2, c2, 1)):
                            ing = cc >= 0
                            nc.tensor.matmul(
                                psT[:, 512 * half:512 * (half + 1)],
                                kT[m][base:base + 64, 128 * jj:128 * (jj + 1)],
                                qb_l, start=True, stop=not ing,
                                skip_group_check=ing)
                            if ing:
                                dg = slice(512 * half + 128 * cc,
                                           512 * half + 128 * (cc + 1))
                                nc.tensor.matmul(
                                    psT[:, dg], identB[:], maskTB[:],
                                    start=False, stop=True,
                                    skip_group_check=True)
                        lo = 128 * c1 if c1 > 0 else 0
                        t2 = t2p.tile([128, 1024], BF16, tag="t2")
                        nc.scalar.activation(
                            t2[:, lo:1024], psT[:, lo:1024], AFT.Exp,
                            bias=0.0, scale=1.0)
                        pend.append((j1, j2, t2, c1, c2))
                        if len(pend) > 2:
                            emit_av(*pend.pop(0))
                        if filler is not None:
                            for _ in range(pulls):
                                nxt = next(filler, None)
                                if nxt is not None:
                                    emit_b1_unit(*nxt)
                    for pv_ in pend:
                        emit_av(*pv_)

                    # normalize by Z2 (row 64 of avp)
                    avr = rzp.tile([128, 512], F32, tag="avr")
                    nc.vector.tensor_copy(avr[0:65, :], avp[0:65, :])
                    rz2 = rzp.tile([128, 512], F32, tag="rz2")
                    nc.vector.reciprocal(rz2[0:1, :], avr[64:65, :])
                    rbc = rzp.tile([128, 512], F32, tag="rbc")
                    nc.gpsimd.partition_broadcast(rbc[0:64, :], rz2[0:1, :])
                    nc.vector.tensor_tensor(
                        out=attT[m][base:base + 64, rcols],
                        in0=avr[0:64, :], in1=rbc[0:64, :], op=ALU.mult)

                def emit_c(t):
                    for rb in range(4 * t, 4 * t + 4):
                        for nn in range(2):
                            ppt = scp.tile([128, 1024], F32, tag="sc")
                            pp = ppt[:, 0:512]
                            for m in range(2):
                                nc.tensor.matmul(
                                    pp[:], attT[m][:, 128 * rb:128 * (rb + 1)],
                                    woS[m][:, 512 * nn:512 * (nn + 1)],
                                    start=(m == 0), stop=(m == 1))
                            ost = ostp.tile([128, 512], F32, tag="ost")
                            nc.vector.tensor_copy(ost[:], pp[:])
                            nc.sync.dma_start(
                                partial[128 * rb:128 * (rb + 1),
                                        512 * nn:512 * (nn + 1)],
                                ost[:])

                # prologue: B1 for row-group 0, v projections interleaved
                for rb in range(4):
                    for h in range(NH):
                        emit_b1_unit(rb, h)
                        emit_v(4 * rb + h)
                emit_stats_qb(0)

                for t in range(NT):
                    units = []
                    if t + 1 < NT:
                        units = [(4 * (t + 1) + j, hh)
                                 for hh in range(NH) for j in range(4)]
                    fl = iter(units)
                    for h in range(NH):
                        emit_b2_head(t, h, fl, pulls=(2 if t == 0 else 1))
                    emit_c(t)
                    for rem in fl:
                        emit_b1_unit(*rem)
                    if t + 1 < NT:
                        emit_stats_qb(t + 1)

    nc.compile()
    return nc


_NC_CACHE = None
_LAST_IN_MAPS = None


def kernel(x, Wq, Wk, Wv, Wo, bo):
    global _NC_CACHE, _LAST_IN_MAPS
    x = np.asarray(x, dtype=np.float32)
    Wq = np.asarray(Wq, dtype=np.float32)
    Wk = np.asarray(Wk, dtype=np.float32)
    Wv = np.asarray(Wv, dtype=np.float32)
    Wo = np.asarray(Wo, dtype=np.float32)
    bo = np.asarray(bo, dtype=np.float32)

    if _NC_CACHE is None:
        _NC_CACHE = build_kernel()
    nc = _NC_CACHE

    woT_full = np.ascontiguousarray(Wo.T)  # [c, o]

    in_maps = []
    for c in range(N_CORES):
        b = c // 4
        s0 = CD * (c % 4)
        sl = slice(s0, s0 + CD)
        in_maps.append({
            "xT": np.ascontiguousarray(x[b].T),
            "wqT": np.ascontiguousarray(Wq[sl, :].T),
            "wkT": np.ascontiguousarray(Wk[sl, :].T),
            "wvT": np.ascontiguousarray(Wv[sl, :].T),
            "woT": np.ascontiguousarray(woT_full[sl, :]),
        })

    _LAST_IN_MAPS = in_maps
    res = run_bass_kernel_spmd(nc, in_maps, core_ids=list(range(N_CORES)))

    out = np.zeros((B, N, DIM), dtype=np.float32)
    for c in range(N_CORES):
        out[c // 4] += res.results[c]["partial"]
    out += bo[None, None, :]
    return out
